# revision 29
# baseline (speedup 1.0000x reference)
"""Trainium2 Bass kernel for a CapsuleNet dynamic-routing layer.

Math (per batch element b):
    u_hat[b,i,o,d] = sum_k W[i,o,d,k] * x[b,i,k]      # B=256, IC=1152, OC=10, OD=16, ID=8
    b_log = 0
    for it in 0..2:
        c = softmax(b_log, axis=o)
        s[b,o,d] = sum_i c[b,i,o] * u_hat[b,i,o,d]
        v = squash(s)
        if it < 2: b_log += sum_d u_hat * v

Sharding: data-parallel over B across 8 cores (32 local rows), W replicated.

Per-core layout: partition axis p = bp*16 + ii (bp = b%8, ii = i%16); the
u_hat build contracts q = ii*8 + k with a host-prepacked block-diagonal x
(lhsT) against the W stack (rhs), one matmul per (iblk, bblk).  u_hat lives in
SBUF as [128(p), 72(iblk), 4(bblk), 160(d*10+o)] fp16 — the capsule axis is
d-MAJOR so every level of the agreement's pairwise d-add tree operates on
long contiguous runs (DVE 2x mode); the o axis (10) stays innermost and is
what softmax/s-extraction index.  b-logits are j-major (blgT) so agreement
writes are natural; the ACT exp absorbs the one transpose into c's o-major
layout.  The c-weighted i-reduction runs on the PE via a block-diagonal c per
bblk accumulating into PSUM over all iblk; all PSUM drains ride the ACT
engine so DVE+gpsimd are free for the agreement, which overlaps the build
(iter 0) and the next b-block's PE matmuls (iter 1).  fp16 everywhere keeps
rounding ~5e-4.
"""

import sys

sys.path.insert(0, "/opt/trn_rl_repo")

from contextlib import ExitStack

import numpy as np

import concourse.bass as bass
import concourse.tile as tile
from concourse import mybir

# fp16 (not bf16): same PE/DVE throughput for 2-byte dtypes, 4 extra mantissa
# bits; all tensors here have magnitudes well inside fp16 normal range.
BF = mybir.dt.float16
F32 = mybir.dt.float32
AX = mybir.AxisListType
AF = mybir.ActivationFunctionType
ALU = mybir.AluOpType

N_CORES = 8
B_FULL, IC, OC, OD, ID = 256, 1152, 10, 16, 8
B_LOC = B_FULL // N_CORES          # 32
NIB = IC // 16                     # 72 i-blocks of 16
NBB = B_LOC // 8                   # 4 b-blocks of 8
F = OC * OD                        # 160

# Agreement runs entirely on DVE: gpsimd TensorTensor is ~4x slower AND
# SBUF contention from a concurrent gpsimd unit halves DVE throughput
# (HW-measured: DVE op median 1840ns with gp overlap vs 908ns without), so
# offloading units to gpsimd is net negative.  gpsimd keeps only the memsets
# and the c-scatter DMA ring.
AGR0_GP_UNITS = set()
AGR1_GP_UNITS = set()


def _squash(nc, smp, ps, scale, vout, od_major_out=True):
    """vout = squash(scale * ps) with ps a [P, 160] psum slab (f32, (d,o)).

    squash(s) = (n^2/(1+n^2)) * s/(n + 1e-8),  n = ||s||_2 over d.
    With n >= ~1e-2 here the 1e-8 is negligible: f = n/(1+n^2).
    od_major_out=False writes vout in (o,d) order (for the DRAM output).
    """
    P = ps.shape[0]
    sv = smp.tile([P, F], BF, tag="sv")
    nc.scalar.copy(sv[:], ps[:])
    sq = smp.tile([P, F], BF, tag="sq")
    nc.vector.tensor_mul(sq[:], sv[:], sv[:])
    # sum over d: in-place pairwise tree on the d-major view (f32, tiny)
    sqv = sq[:].rearrange("p (d o) -> p d o", o=OC)
    nc.vector.tensor_add(sqv[:, 0:8, :], sqv[:, 0:8, :], sqv[:, 8:16, :])
    nc.vector.tensor_add(sqv[:, 0:4, :], sqv[:, 0:4, :], sqv[:, 4:8, :])
    nc.vector.tensor_add(sqv[:, 0:2, :], sqv[:, 0:2, :], sqv[:, 2:4, :])
    n2 = smp.tile([P, OC], F32, tag="n2")
    if scale != 1.0:
        nc.vector.tensor_add(n2[:], sqv[:, 0, :], sqv[:, 1, :])
        nc.vector.tensor_scalar_mul(n2[:], n2[:], float(scale) ** 2)
    else:
        nc.vector.tensor_add(n2[:], sqv[:, 0, :], sqv[:, 1, :])
    n1 = smp.tile([P, OC], F32, tag="n1")
    nc.vector.tensor_scalar_add(n1[:], n2[:], 1.0)
    sn = smp.tile([P, OC], F32, tag="sn")
    nc.scalar.sqrt(sn[:], n2[:])          # ACT, parallel with the DVE recip
    r1 = smp.tile([P, OC], F32, tag="r1")
    nc.vector.reciprocal(r1[:], n1[:])
    f1 = smp.tile([P, OC], F32, tag="f1")
    nc.vector.tensor_mul(f1[:], sn[:], r1[:])
    if scale != 1.0:
        nc.vector.tensor_scalar_mul(f1[:], f1[:], float(scale))
    if od_major_out:
        dst = vout[:].rearrange("p (d o) -> p d o", o=OC)
    else:
        dst = vout[:].rearrange("p (o d) -> p o d", d=OD).transpose([0, 2, 1])
    nc.vector.tensor_mul(
        dst,
        sv[:].rearrange("p (d o) -> p d o", o=OC),
        f1[:].unsqueeze(1).broadcast_to((P, OD, OC)))


def _split_multiwait(nc):
    """The walrus in this container encodes at most ONE semaphore wait on
    Matmult/Ldweights and HWDGE DMACopy instructions ("Too many sync wait
    commands").  Hoist excess waits onto same-engine NoOps placed directly
    before the instruction - position-identical semantics, ~2 cycles each.
    SWDGE (Pool-queue) DMAs handle multi-waits fine and are left alone.
    """
    for fn in nc.m.functions:
        for bb in fn.blocks:
            out = []
            k = 0
            for ins in bb.instructions:
                si = ins.sync_info
                waits = list(si.on_wait) if si is not None and si.on_wait else []
                limit = 1
                if ins.opcode == "DMACopy":
                    q = str(getattr(ins, "queue", "") or "")
                    if "HW" in q and len(waits) > 1:
                        # HWDGE descriptors encode one wait and can't be gated
                        # by engine NoOps.  Tile's redundant-wait optimizer is
                        # disabled (inc-6505), so these DMAs carry DMAHW-queue
                        # sems alongside the real engine-sem dep.  Here every
                        # such DMAHW wait is either dominated by the kept
                        # engine wait (the engine's last read of the reused
                        # buffer transitively waited the old DMA's queue sems)
                        # or orders against a disjoint-memory DMA; drop them.
                        eng_w = [w for w in waits if "DMAHW" not in w.ant_name]
                        if len(eng_w) != 1:
                            raise AssertionError(
                                f"HWDGE DMA {ins.name}: can't reduce waits "
                                f"{[w.ant_name for w in waits]} to one")
                        ins.sync_info = mybir.SyncInfo(
                            on_wait=eng_w,
                            on_update=list(si.on_update) if si.on_update else [])
                        out.append(ins)
                        continue
                if len(waits) > limit:
                    for w in waits[:-limit]:
                        nop = mybir.InstNoOp(name=f"{ins.name}-wn{k}", ins=[], outs=[])
                        k += 1
                        nop.engine = ins.engine
                        nop.sync_info = mybir.SyncInfo(on_wait=[w], on_update=[])
                        out.append(nop)
                    ins.sync_info = mybir.SyncInfo(
                        on_wait=waits[-limit:],
                        on_update=list(si.on_update) if si.on_update else [])
                out.append(ins)
            bb.instructions = out


def build_program(split_waits=True):
    """split_waits=True applies the walrus 1-wait workaround (required for
    hardware compiles); CoreSim/TimelineSim need the unsplit program."""
    nc = bass.Bass()
    bd_d = nc.declare_dram_parameter("bd", [8, 128, 9, NBB, 128], BF, isOutput=False)
    xt_d = nc.declare_dram_parameter("xt", [128, NIB, B_LOC], BF, isOutput=False)
    ws_d = nc.declare_dram_parameter("ws", [8, 128, 9, F], BF, isOutput=False)
    msk_d = nc.declare_dram_parameter("msk", [80, F], BF, isOutput=False)
    o80_d = nc.declare_dram_parameter("o80", [80, 8], BF, isOutput=False)
    sel_d = nc.declare_dram_parameter("sel", [32, NBB, 128], BF, isOutput=False)
    out_d = nc.declare_dram_parameter("out", [B_LOC, F], F32, isOutput=True)

    with ExitStack() as ctx:
        tc = ctx.enter_context(tile.TileContext(nc))
        st = ctx.enter_context(tc.tile_pool(name="st", bufs=1))
        bdp = ctx.enter_context(tc.tile_pool(name="bdp", bufs=2))
        y2p = ctx.enter_context(tc.tile_pool(name="y2p", bufs=3))
        tsp = ctx.enter_context(tc.tile_pool(name="tsp", bufs=4))
        mkp = ctx.enter_context(tc.tile_pool(name="mkp", bufs=2))
        # all four per-bblk vx broadcasts live across a whole iteration
        vxp = ctx.enter_context(tc.tile_pool(name="vxp", bufs=6))
        smp = ctx.enter_context(tc.tile_pool(name="smp", bufs=3))
        pbig = ctx.enter_context(tc.tile_pool(name="pbig", bufs=4, space="PSUM"))
        psml = ctx.enter_context(tc.tile_pool(name="psml", bufs=4, space="PSUM"))

        # --- persistent tiles ---
        u_hat = st.tile([128, NIB, NBB, F], BF, tag="u_hat")
        ws_sb = st.tile([128, 8, 9, F], BF, tag="ws_sb")
        blgT = st.tile([128, NBB, NIB, OC], BF, tag="blgT")
        c_sb = st.tile([128, NBB, OC, NIB], BF, tag="c_sb")
        cb0 = st.tile([128, 80, NIB], BF, tag="cb0")
        cb1 = st.tile([128, 80, NIB], BF, tag="cb1")
        msk_sb = st.tile([80, F], BF, tag="msk_sb")
        o80_sb = st.tile([80, 8], BF, tag="o80_sb")
        sel_sb = st.tile([32, NBB, 128], BF, tag="sel_sb")
        xt_sb = st.tile([128, NIB, B_LOC], BF, tag="xt_sb")
        v32 = st.tile([32, F], BF, tag="v32")
        v8 = [st.tile([8, F], BF, tag=f"v8_{i}", name=f"v8_{i}") for i in range(NBB)]
        of8 = [st.tile([8, F], F32, tag=f"of8_{i}", name=f"of8_{i}") for i in range(NBB)]

        # --- input loads (both HWDGE queues) + zero-init of the c-blockdiag ---
        nc.sync.dma_start(out=msk_sb[:], in_=msk_d[:])
        nc.sync.dma_start(out=o80_sb[:], in_=o80_d[:])
        nc.sync.dma_start(out=sel_sb[:], in_=sel_d[:])
        nc.scalar.dma_start(out=xt_sb[:], in_=xt_d[:])
        # ws: [8(e), 128(q), 9, F] -> [128, 8, 9, F]; one DMA per e on
        # alternating HWDGE queues so pass-1 starts after the first slice
        for e in range(8):
            eng = (nc.scalar, nc.sync)[e % 2]
            eng.dma_start(out=ws_sb[:, e], in_=ws_d[e])
        nc.gpsimd.memset(cb0[:], 0.0)
        nc.gpsimd.memset(cb1[:], 0.0)

        def agr_unit(bblk, lo, hi, eng, first, vx):
            """blgT[:, bblk, lo:hi, :] (+)= sum_d u_hat*vx for an iblk unit."""
            n = hi - lo
            sl = slice(lo, hi)
            y2 = y2p.tile([128, 24, F], BF, tag="y2")
            eng.tensor_mul(
                y2[:, :n, :], u_hat[:, sl, bblk, :],
                vx[:].unsqueeze(1).broadcast_to((128, n, F)))
            # in-place pairwise-add tree over d-major halves (fp16, 2x mode,
            # contiguous runs of 80/40/20/10 at every level)
            y2v = y2[:, :n, :].rearrange("p j (d o) -> p j d o", o=OC)
            eng.tensor_add(
                y2v[:, :, 0:8, :], y2v[:, :, 0:8, :], y2v[:, :, 8:16, :])
            eng.tensor_add(
                y2v[:, :, 0:4, :], y2v[:, :, 0:4, :], y2v[:, :, 4:8, :])
            eng.tensor_add(
                y2v[:, :, 0:2, :], y2v[:, :, 0:2, :], y2v[:, :, 2:4, :])
            dst = blgT[:, bblk, sl, :]
            if first:
                eng.tensor_add(dst, y2v[:, :, 0, :], y2v[:, :, 1, :])
            else:
                ts = tsp.tile([128, 24, OC], BF, tag="ts")
                eng.tensor_add(ts[:, :n, :], y2v[:, :, 0, :], y2v[:, :, 1, :])
                eng.tensor_add(dst, dst, ts[:, :n, :])

        def make_vx(bblk, vt, nr):
            """Broadcast v rows of bblk across its (bp, ii) partitions."""
            pvx = psml.tile([128, F], F32, tag="psml", name=f"pvx{bblk}")
            nc.tensor.matmul(
                pvx[:], lhsT=sel_sb[0:nr, bblk if nr == 32 else 0, :],
                rhs=vt[0:nr, :], start=True, stop=True)
            vx = vxp.tile([128, F], BF, tag="vx", name=f"vx{bblk}")
            nc.scalar.copy(vx[:], pvx[:])
            return vx

        def softmax_bblk(bblk):
            # softmax over o for this b-block (no max-sub: |logits| << 1).
            # exp reads j-major logits and writes o-major c (ACT pays the
            # transpose); sum over o = pairwise tree on contiguous j-runs.
            cs = c_sb[:, bblk, :, :]                        # [p, o, j]
            nc.scalar.activation(
                cs.transpose([0, 2, 1]), blgT[:, bblk, :, :], AF.Exp)
            s5 = smp.tile([128, 5, NIB], BF, tag="s5")
            nc.vector.tensor_add(s5[:], cs[0:128, 0:5, :], cs[0:128, 5:10, :])
            nc.vector.tensor_add(
                s5[:, 0:2, :], s5[:, 0:2, :], s5[:, 2:4, :])
            nc.vector.tensor_add(s5[:, 0, :], s5[:, 0, :], s5[:, 1, :])
            sm = smp.tile([128, NIB], F32, tag="sm")
            nc.vector.tensor_add(sm[:], s5[:, 0, :], s5[:, 4, :])
            rr = smp.tile([128, NIB], F32, tag="rr")
            nc.vector.reciprocal(rr[:], sm[:])
            nc.vector.tensor_mul(
                cs, cs, rr[:].unsqueeze(1).broadcast_to((128, OC, NIB)))

        def scatter_bblk(bblk):
            # c into its block-diagonal lhsT slots (SWDGE: multi-wait is fine)
            cbt = (cb0, cb1)[bblk % 2]
            for bp in range(8):
                nc.gpsimd.dma_start(
                    out=cbt[bp * 16:(bp + 1) * 16, bp * 10:(bp + 1) * 10, :],
                    in_=c_sb[bp * 16:(bp + 1) * 16, bblk, :, :])
            return cbt

        def sstep_mms(bblk, cbt, lo=0, hi=NIB, ps1=None):
            # s-stage 1: diag(C^T U) via blockdiag-c matmuls over iblk lo:hi
            if ps1 is None:
                ps1 = pbig.tile([80, F], F32, tag="pbig", name=f"ps1_{bblk}")
            for j in range(lo, hi):
                nc.tensor.matmul(
                    ps1[:], lhsT=cbt[:, :, j], rhs=u_hat[:, j, bblk, :],
                    start=(j == 0), stop=(j == NIB - 1))
            return ps1

        def sstep_extract(ps1):
            # s-stage 2: o-mask then tiny reduce MM -> s [8, F] psum
            mskd = mkp.tile([80, F], BF, tag="mskd")
            nc.vector.tensor_mul(mskd[:], ps1[:], msk_sb[:])
            psv = psml.tile([8, F], F32, tag="psml")
            nc.tensor.matmul(
                psv[:], lhsT=o80_sb[:], rhs=mskd[:], start=True, stop=True)
            return psv

        def sstep_bblk(bblk, cbt):
            return sstep_extract(sstep_mms(bblk, cbt))

        # --- pass 1: iter-0 uniform-c reduction s0 = 0.1*sum_i u_hat computed
        # directly as x @ W over the full (i,k) contraction from xt/ws -- 72
        # m=32 matmuls into one [32,160] psum, then one batched [32,160]
        # squash.  Emitted first so iter-0's agreement can overlap the build.
        ps0 = psml.tile([32, F], F32, tag="psml", name="ps0")
        for e in range(8):
            for j in range(9):
                iblk = e * 9 + j
                nc.tensor.matmul(
                    ps0[:], lhsT=xt_sb[:, iblk, :], rhs=ws_sb[:, e, j, :],
                    start=(iblk == 0), stop=(iblk == NIB - 1))
        _squash(nc, smp, ps0[:], 0.1, v32)

        # --- pass 2: build u_hat, one matmul per (iblk, bblk), K=128=(ii,k),
        # N=160; every PSUM drain rides ACT so DVE+gpsimd are free for the
        # iter-0 agreement units emitted after each e-pair's drains. ---
        vx0 = [None] * NBB
        for e in range(8):
            bdt = bdp.tile([128, 9, NBB, 128], BF, tag="bdt")
            nc.sync.dma_start(out=bdt[:], in_=bd_d[e])
            for j in range(9):
                for h in range(2):
                    ps = pbig.tile([128, 2, F], F32, tag="pbig")
                    for bb in range(2):
                        nc.tensor.matmul(
                            ps[:, bb, :], lhsT=bdt[:, j, h * 2 + bb, :],
                            rhs=ws_sb[:, e, j, :], start=True, stop=True)
                    eng = nc.vector if e == 0 else nc.scalar
                    eng.copy(u_hat[:, e * 9 + j, h * 2:(h + 1) * 2, :], ps[:]) \
                        if e != 0 else nc.vector.tensor_copy(
                            u_hat[:, e * 9 + j, h * 2:(h + 1) * 2, :], ps[:])
            if e == 0:
                for bblk in range(NBB):
                    vx0[bblk] = make_vx(bblk, v32, 32)
            if e % 2 == 1 and e < 7:
                ep = e // 2
                for bblk in range(NBB):
                    agr_unit(bblk, ep * 18, (ep + 1) * 18, nc.vector, True,
                             vx0[bblk])
            if e == 6:
                for bblk in range(NBB):
                    agr_unit(bblk, 54, 63, nc.vector, True, vx0[bblk])

        # --- iters 1+2, software-pipelined.  Per bblk the serial chain is
        # agr1(b) -> softmax2(b) -> scatter2(b) -> sstep2(b); emission places
        # each bblk's DVE-side work (squash, agreement, softmax) one sstep
        # behind the PE's matmul chains so neither engine waits on the other:
        # the iter-0 tail units fill DVE while the first ssteps run, and pvx
        # matmuls sit late enough in the PE stream that the squash chain has
        # always resolved.  cb0 serves b0/b2, cb1 b1/b3: each scatter is
        # emitted only after its tile's previous reader (program order =
        # Tile's dependency order; earlier would alias the c values). ---
        def agr0_tail(bblk):
            agr_unit(bblk, 63, 72, nc.vector, True, vx0[bblk])
            softmax_bblk(bblk)

        def squash1(bblk, psv):
            _squash(nc, smp, psv, 1.0, v8[bblk])

        def sstep2_out(bblk):
            psv = sstep_bblk(bblk, (cb0, cb1)[bblk % 2])
            _squash(nc, smp, psv, 1.0, of8[bblk], od_major_out=False)
            nc.scalar.dma_start(
                out=out_d[bblk * 8:(bblk + 1) * 8, :], in_=of8[bblk][:])

        agr0_tail(0)
        scatter_bblk(0)
        agr0_tail(1)
        scatter_bblk(1)

        def agr1u(bblk, u, vx):
            agr_unit(bblk, u * 24, (u + 1) * 24, nc.vector, False, vx)

        def extract_squash1(bblk, pm):
            psv = sstep_extract(pm)
            _squash(nc, smp, psv, 1.0, v8[bblk])

        def extract_out(bblk, pm):
            psv = sstep_extract(pm)
            _squash(nc, smp, psv, 1.0, of8[bblk], od_major_out=False)
            nc.scalar.dma_start(
                out=out_d[bblk * 8:(bblk + 1) * 8, :], in_=of8[bblk][:])

        vx1 = [None] * NBB
        agr0_tail(2)
        agr0_tail(3)
        pm0 = sstep_mms(0, cb0)
        psv0 = extract_squash1(0, pm0)
        pm1 = sstep_mms(1, cb1, 0, 36)
        vx1[0] = make_vx(0, v8[0], 8)     # pvx rides the half-chain boundary
        sstep_mms(1, cb1, 36, NIB, ps1=pm1)
        agr1u(0, 0, vx1[0])
        agr1u(0, 1, vx1[0])
        extract_squash1(1, pm1)
        scatter_bblk(2)
        scatter_bblk(3)
        agr1u(0, 2, vx1[0])
        softmax_bblk(0)
        pm2 = sstep_mms(2, cb0, 0, 36)
        vx1[1] = make_vx(1, v8[1], 8)
        sstep_mms(2, cb0, 36, NIB, ps1=pm2)
        agr1u(1, 0, vx1[1])
        agr1u(1, 1, vx1[1])
        extract_squash1(2, pm2)
        scatter_bblk(0)          # iter-2 c for b0 (sm2(0) done, cb0 free)
        agr1u(1, 2, vx1[1])
        softmax_bblk(1)
        pm3 = sstep_mms(3, cb1, 0, 36)
        vx1[2] = make_vx(2, v8[2], 8)
        sstep_mms(3, cb1, 36, NIB, ps1=pm3)
        agr1u(2, 0, vx1[2])
        agr1u(2, 1, vx1[2])
        extract_squash1(3, pm3)
        scatter_bblk(1)          # after sstep1(3) read cb1, sm2(1) done
        agr1u(2, 2, vx1[2])
        softmax_bblk(2)
        pm20 = sstep_mms(0, cb0, 0, 36)
        vx1[3] = make_vx(3, v8[3], 8)
        sstep_mms(0, cb0, 36, NIB, ps1=pm20)
        agr1u(3, 0, vx1[3])
        agr1u(3, 1, vx1[3])
        extract_out(0, pm20)
        scatter_bblk(2)          # after sstep2(0) read cb0, sm2(2) done
        agr1u(3, 2, vx1[3])
        softmax_bblk(3)
        pm21 = sstep_mms(1, cb1)
        extract_out(1, pm21)
        scatter_bblk(3)          # after sstep2(1) read cb1, sm2(3) done
        pm22 = sstep_mms(2, cb0)
        extract_out(2, pm22)
        pm23 = sstep_mms(3, cb1)
        extract_out(3, pm23)
    if split_waits:
        _split_multiwait(nc)
    return nc


def _host_inputs(x, W):
    """Per-core input maps from full x [256,1152,8] f32, W [1,1152,10,16,8] f32."""
    bf = np.float16
    W0 = np.asarray(W[0], dtype=np.float32)
    # ws[e, q=(ii,k), j, (d,o)] = W[(e*9+j)*16+ii, o, d, k]   (d-major F axis)
    ws = np.ascontiguousarray(
        W0.reshape(8, 9, 16, OC, OD, ID).transpose(0, 2, 5, 1, 4, 3)
        .reshape(8, 128, 9, F)).astype(bf)
    msk = np.zeros((80, F), dtype=bf)
    for bpp in range(8):
        for o in range(OC):
            msk[bpp * 10 + o, o::OC] = 1.0
    o80 = np.zeros((80, 8), dtype=bf)
    for p in range(80):
        o80[p, p // 10] = 1.0
    # sel[q, b, p] = 1 iff q == 8*b + p//16  (broadcast v-row of b-block b's
    # bp to the 16 partitions (bp, ii); [0:8, 0, :] doubles as the v8 form)
    sel = np.zeros((32, NBB, 128), dtype=bf)
    for b in range(NBB):
        for p in range(128):
            sel[8 * b + p // 16, b, p] = 1.0

    in_maps = []
    for c in range(N_CORES):
        xc = np.asarray(x[c * B_LOC:(c + 1) * B_LOC], dtype=np.float32)
        # bd[e, q=(ii,k), j, bb, m=(bp,ii')] = x[bb*8+bp, (e*9+j)*16+ii, k] iff ii'==ii
        r = xc.reshape(NBB, 8, 8, 9, 16, ID)          # [bb, bp, e, j, ii, k]
        bd6 = np.zeros((8, 16, ID, 9, NBB, 8, 16), dtype=np.float32)
        for ii in range(16):
            # [bb, bp, e, j, k] -> [e, k, j, bb, bp]
            bd6[:, ii, :, :, :, :, ii] = r[:, :, :, :, ii, :].transpose(2, 4, 3, 0, 1)
        bd = np.ascontiguousarray(bd6.reshape(8, 128, 9, NBB, 128)).astype(bf)
        # xt[q=(ii,k), iblk, b] = x[b, iblk*16+ii, k]
        xt = np.ascontiguousarray(
            xc.reshape(B_LOC, NIB, 16, ID).transpose(2, 3, 1, 0)
            .reshape(128, NIB, B_LOC)).astype(bf)
        in_maps.append(
            {"bd": bd, "xt": xt, "ws": ws, "msk": msk, "o80": o80, "sel": sel})
    return in_maps


_NC_CACHE = {}


def kernel(x, W):
    from concourse.bass_utils import run_bass_kernel_spmd

    if "nc" not in _NC_CACHE:
        _NC_CACHE["nc"] = build_program()
    nc = _NC_CACHE["nc"]
    in_maps = _host_inputs(x, W)
    res = run_bass_kernel_spmd(nc, in_maps, core_ids=list(range(N_CORES)))
    out = np.concatenate([r["out"] for r in res.results], axis=0)
    return out.reshape(B_FULL, OC, OD).astype(np.float32)


if __name__ == "__main__":
    nc = build_program()
    print("program built ok")
